# revision 1
# baseline (speedup 1.0000x reference)
"""Multi-head cross-attention (self-attention variant) on 8 Trainium2 NeuronCores.

Problem: x[1,4096,1024]; Wq/Wk/Wv[1024,1024] -> 16 heads x 64 dim; softmax(QK^T/8)V;
merge heads; @ Wo + bo -> [1,4096,1024].

Sharding: tensor-parallel over heads. Core k owns heads (2k, 2k+1) = inner cols
[128k : 128k+128]. All activations/weights are bf16 (measured rel err ~9e-3 vs
the 2e-2 gate), which keeps every matmul at 1 PE cycle/row at any free size.

Per core:
  - Q^T/K^T [128, 4096] projected chunk-wise (contraction = model dim, moving = x^T).
  - V projected directly in [j, d] layout (stationary = x^T tile, moving = Wv);
    4 j-block slots share one PSUM bank as a single accumulation group (a
    start=True matmul pend-zeroes the whole 2KB bank; later start=False slots
    land on pending-zero bytes and accumulate from zero).
  - Scores S^T[j, i] per (head, j-block): stat = K^T block, mov = Q^T; each
    [128, 512] f32 output exactly fills one PSUM bank. Six banks rotate as
    score buffers -- the deep window keeps the PE->exp->PE loop from being
    latency-bound (2-bank tiles with a 3-deep window cost ~25% throughput).
  - exp: softmax without max-subtraction (logits ~ N(0,1), exp is safe).
    Split 34/30 between the scalar engine (activation Exp, bf16 out) and DVE
    (Schraudolph fast exp2: bf16(exp(s)) bits == int16(s*FE_A + FE_B), one
    fused tensor_scalar writing int16 -- the DVE write-converter truncates,
    centered by +0.5 in FE_B; int16 range is safe for |logit| << 64 sigma).
    GPSIMD cannot read PSUM (BIR verifier), so it takes no exp work.
  - PV transposed: stat = P^T block [j, 128 i], mov = V [j, 64 d] -> O[i, d],
    64 rows/matmul instead of 512 (2x fewer PE rows than O^T = V^T P). All 8
    (i-block, head) slots accumulate in ONE bank as a single group; row-sums
    accumulate in a second bank via 1-row matmuls against a ones vector.
  - Normalize: DVE reciprocal of the 8 sums, then ONE broadcast tensor_tensor
    multiply (stride-0 AP spreads rcp[i, slot] over each slot's 64 dims).
  - O [i, d] -> O^T via DMA xbar transpose (dma_start_transpose: no PE, PSUM
    or DVE cost). Two half-width AllToAlls reshard head-parallel ->
    sequence-parallel (the second half's exchange overlaps the first half's
    go loads and output-proj matmuls); core k ends with rows [512k : 512k+512]
    of the merged-head activation and applies the full Wo; host concatenates
    row slices and adds bo.

Emission order software-pipelines the in-order PE queue: chunk c's scores
interleave with chunk c-1's PV at 2 PV-parts per j-block for the first half
of the chunk (PV finishes mid-chunk, giving normalize a long window before
the accumulator bank is reused); chunk 7's own PV runs in its second half.
Projections for chunks 1-7 are woven into chunk 0's score stream.
"""
import numpy as np
from contextlib import ExitStack

N_CORES = 8
N = 4096          # sequence length
QD = 1024         # model dim
DH = 64           # head dim
HPC = 2           # heads per core
CPC = HPC * DH    # inner cols per core = 128
IC = 512          # i-chunk (query) size
NI = N // IC      # 8 chunks
JB = 128          # j-block (key) size
NJ = N // JB      # 32 blocks
NG = 16           # j-groups per chunk (2 j-blocks each)
SCALE = DH ** -0.5

# fast-exp routing per chunk: r = 2*jb + h in [0, 64). Fast tiles run ONE
# fused DVE op: tensor_scalar(s*A + B) written straight to int16 (the DVE
# write-converter truncates, which the +0.5 in FE_B centers); the rest use
# the scalar engine's Exp. GPSIMD cannot touch PSUM, so it takes no exp work.
FE_DVE = frozenset(r for r in range(64) if r % 2 == 1) - {15, 47}
# last chunk: r=62 stays on Act so the final two exps (r=62 Act, r=63 DVE)
# drain in parallel rather than serializing on DVE
FE_DVE_LAST = FE_DVE
# bf16 bits of exp(s*SCALE) ~= int16(s*FE_A + FE_B):
#   FE_A = 2^23/(ln2 * 2^16) * SCALE,  FE_B = 127*128 - C/2^16 (+0.5 trunc bias)
FE_A = 184.6638356 * SCALE
FE_B = 16249.066

_CACHE = {}


def _build(single=False):
    from concourse import bacc, tile, mybir

    f32 = mybir.dt.float32
    bf16 = mybir.dt.bfloat16
    i16 = mybir.dt.int16
    Exp = mybir.ActivationFunctionType.Exp
    Mult = mybir.AluOpType.mult
    Add = mybir.AluOpType.add
    Div = mybir.AluOpType.divide

    nc = bacc.Bacc("TRN2", target_bir_lowering=False, debug=False,
                   enable_asserts=False, num_devices=1 if single else N_CORES)

    xt_d = nc.dram_tensor("xt", [QD, N], bf16, kind="ExternalInput").ap()
    # wq/wk/wv arrive host-pre-arranged in SBUF layout [p, (t c)] so the
    # startup loads are contiguous 2KB-row copies (256B runs pay a 2x DMA
    # latency multiplier)
    wq_d = nc.dram_tensor("wq", [128, 8 * CPC], bf16, kind="ExternalInput").ap()
    wk_d = nc.dram_tensor("wk", [128, 8 * CPC], bf16, kind="ExternalInput").ap()
    wv_d = nc.dram_tensor("wv", [128, 8 * CPC], bf16, kind="ExternalInput").ap()
    wo_d = nc.dram_tensor("wo", [QD, QD], bf16, kind="ExternalInput").ap()
    y_d = nc.dram_tensor("y_out", [IC, QD], bf16, kind="ExternalOutput").ap()

    with tile.TileContext(nc) as tc:
        with ExitStack() as ctx:
            sb = ctx.enter_context(tc.tile_pool(name="sb", bufs=1))
            xt_pool = ctx.enter_context(tc.tile_pool(name="xt", bufs=3))
            pt_pool = ctx.enter_context(tc.tile_pool(name="pt", bufs=76))
            o_pool = ctx.enter_context(tc.tile_pool(name="osb", bufs=3))
            ot_pool = ctx.enter_context(tc.tile_pool(name="ot", bufs=2))
            sm_pool = ctx.enter_context(tc.tile_pool(name="sm", bufs=2))
            y_pool = ctx.enter_context(tc.tile_pool(name="ysb", bufs=8))
            sc_ps = ctx.enter_context(tc.tile_pool(name="sc", bufs=6, space="PSUM"))
            aux_ps = ctx.enter_context(tc.tile_pool(name="aux", bufs=2, space="PSUM"))
            dram = ctx.enter_context(tc.tile_pool(name="dram", bufs=1, space="DRAM"))

            # --- static SBUF residents ---
            qts = [sb.tile([CPC, IC], bf16, name=f"qt{c}") for c in range(NI)]
            kts = [sb.tile([CPC, IC], bf16, name=f"kt{c}") for c in range(NI)]
            # vs[c]: V[j, d] for j-block 4c+b at cols [128b : 128b+128]
            vs = [sb.tile([128, IC], bf16, name=f"v{c}") for c in range(NI)]
            wq_sb = sb.tile([128, 8 * CPC], bf16)   # qd-tile t at cols 128t
            wk_sb = sb.tile([128, 8 * CPC], bf16)
            wv_sb = sb.tile([128, 8 * CPC], bf16)
            wo_sb = sb.tile([128, 8 * QD], bf16)    # qd-tile t at cols 1024t
            go_sb = sb.tile([128, 8 * IC], bf16)    # a2a result, r-block at 512r
            ones_sb = sb.tile([128, 1], bf16)

            # reshard buffers split into two contiguous half-width tensors
            # (the collective requires contiguous APs); the second half's
            # exchange overlaps the first half's go loads and output proj
            a2a_in = [dram.tile([N_CORES * CPC, IC // 2], bf16,
                                name=f"a2a_in{u}") for u in range(2)]
            a2a_out = [dram.tile([N_CORES * CPC, IC // 2], bf16,
                                 name=f"a2a_out{u}") for u in range(2)]

            nc.vector.memset(ones_sb[:, :], 1.0)

            def load_w(dst, src):
                nc.sync.dma_start(out=dst[:, :], in_=src)

            def load_wo(dst, src):
                nc.sync.dma_start(
                    out=dst[:, :].rearrange("p (t c) -> p t c", t=8),
                    in_=src.rearrange("(t p) c -> p t c", t=8))

            def load_xt(c):
                xt_c = xt_pool.tile([128, 8 * IC], bf16, name=f"xt{c}", tag="xt")
                nc.sync.dma_start(
                    out=xt_c[:, :].rearrange("p (t i) -> p t i", t=8),
                    in_=xt_d.rearrange("(t p) n -> p t n", t=8)[:, :, IC * c:IC * (c + 1)])
                return xt_c

            def proj_qk(c, xt_c):
                q_ps = aux_ps.tile([128, IC], f32, name="q_ps", tag="aux")
                for t in range(8):
                    nc.tensor.matmul(q_ps[:, :], wq_sb[:, CPC * t:CPC * (t + 1)],
                                     xt_c[:, IC * t:IC * (t + 1)],
                                     start=(t == 0), stop=(t == 7))
                nc.vector.tensor_copy(qts[c][:, :], q_ps[:, :])
                k_ps = aux_ps.tile([128, IC], f32, name="k_ps", tag="aux")
                for t in range(8):
                    nc.tensor.matmul(k_ps[:, :], wk_sb[:, CPC * t:CPC * (t + 1)],
                                     xt_c[:, IC * t:IC * (t + 1)],
                                     start=(t == 0), stop=(t == 7))
                nc.vector.tensor_copy(kts[c][:, :], k_ps[:, :])

            def proj_v(c, xt_c):
                # V in [j, d]: stat = x^T tile (qd x j), mov = Wv tile (qd x d);
                # 4 j-block slots in one bank, single accumulation group
                v_ps = aux_ps.tile([128, IC], f32, name="v_ps", tag="aux")
                for b in range(4):
                    for t in range(8):
                        nc.tensor.matmul(
                            v_ps[:, JB * b:JB * (b + 1)],
                            xt_c[:, IC * t + JB * b:IC * t + JB * (b + 1)],
                            wv_sb[:, CPC * t:CPC * (t + 1)],
                            start=(b == 0 and t == 0), stop=(b == 3 and t == 7))
                nc.vector.tensor_copy(vs[c][:, :], v_ps[:, :])

            def scores_exp(c, jb, h):
                r = 2 * jb + h
                fe_dve = FE_DVE_LAST if c == NI - 1 else FE_DVE
                s_ps = sc_ps.tile([128, IC], f32, name="s_ps", tag="sc")
                nc.tensor.matmul(
                    s_ps[:, :],
                    kts[jb // 4][DH * h:DH * (h + 1),
                                 JB * (jb % 4):JB * (jb % 4 + 1)],
                    qts[c][DH * h:DH * (h + 1), :], start=True, stop=True)
                pt = pt_pool.tile([128, IC], bf16, name="pt", tag="pt")
                if r in fe_dve:
                    nc.vector.tensor_scalar(out=pt[:, :].bitcast(i16),
                                            in0=s_ps[:, :],
                                            scalar1=float(FE_A),
                                            scalar2=float(FE_B),
                                            op0=Mult, op1=Add)
                else:
                    nc.scalar.activation(pt[:, :], s_ps[:, :], Exp, scale=SCALE)
                return pt

            def pv_part(jb, pts_c, acc, sums):
                # consume pt tiles of j-block jb: O[i, d] and sum rows
                g4, b = jb // 4, jb % 4
                for ib in range(4):
                    for h in range(HPC):
                        first = (jb == 0 and ib == 0 and h == 0)
                        last = (jb == NJ - 1 and ib == 3 and h == 1)
                        stat = pts_c[(h, jb)][:, JB * ib:JB * (ib + 1)]
                        s = 2 * ib + h
                        nc.tensor.matmul(
                            acc[:, DH * s:DH * (s + 1)], stat,
                            vs[g4][:, JB * b + DH * h:JB * b + DH * (h + 1)],
                            start=first, stop=last)
                        nc.tensor.matmul(
                            sums[:, s:s + 1], stat, ones_sb[:, :],
                            start=first, stop=last)

            def norm_transpose(c, acc, sums):
                # O * (1/sum(exp)) on DVE (the only vector engine allowed to
                # read PSUM): reciprocal of the 8 sums, then per-partition
                # scalar multiplies
                rcp = sm_pool.tile([128, 8], f32, name="rcp", tag="sm")
                nc.vector.reciprocal(rcp[:, :], sums[:, 0:8])
                otT = ot_pool.tile([128, IC], bf16, name="otT", tag="ot")
                # one broadcast multiply normalizes all 8 (i-block, head)
                # slots: rcp[i, s] spreads over each slot's 64 dims via a
                # stride-0 AP
                o_sb = o_pool.tile([128, IC], bf16, name="osb", tag="osb")
                nc.vector.tensor_mul(
                    o_sb[:, :].rearrange("p (s d) -> p s d", s=8),
                    acc[:, :].rearrange("p (s d) -> p s d", s=8),
                    rcp[:, :].to_broadcast([128, 8, DH]))
                for ib in range(4):
                    nc.sync.dma_start_transpose(
                        out=otT[:, JB * ib:JB * (ib + 1)],
                        in_=o_sb[:, JB * ib:JB * (ib + 1)])
                for u in range(2):
                    nc.sync.dma_start(
                        out=a2a_in[u][CPC * c:CPC * (c + 1), :],
                        in_=otT[:, IC // 2 * u:IC // 2 * (u + 1)])

            # --- emission ---
            load_w(wq_sb, wq_d)
            xt0 = xt_pool.tile([128, 8 * IC], bf16, name="xt0", tag="xt")
            for t in range(8):
                nc.sync.dma_start(
                    out=xt0[:, IC * t:IC * (t + 1)],
                    in_=xt_d[128 * t:128 * (t + 1), 0:IC])
            load_w(wk_sb, wk_d)
            load_w(wv_sb, wv_d)
            proj_qk(0, xt0)
            proj_v(0, xt0)

            pts = [dict() for _ in range(NI)]
            # chunk 0 scores interleaved with remaining projections; V lags
            # Q/K by two j-blocks so its PSUM slot reuse never stalls PE
            xts = {}
            for jb in range(NJ):
                if jb % 4 == 0 and jb // 4 + 1 < NI:
                    m = jb // 4 + 1
                    xts[m] = load_xt(m)
                    proj_qk(m, xts[m])
                if jb % 4 == 2 and jb // 4 + 1 < NI:
                    m = jb // 4 + 1
                    proj_v(m, xts[m])
                    del xts[m]
                for h in range(HPC):
                    pts[0][(h, jb)] = scores_exp(0, jb, h)
            load_wo(wo_sb, wo_d)
            # steady: scores(c) interleave with PV(c-1) at double rate in the
            # first half of each chunk, so normalize(c-1) (gpsimd) has a long
            # window before PV(c) reuses the accumulator bank. Chunk 7's own
            # PV runs in its second half, right behind its exps.
            for c in range(1, NI):
                acc = aux_ps.tile([128, IC], f32, name="acc", tag="aux")
                sums = aux_ps.tile([128, 16], f32, name="sums", tag="aux")
                for jb in range(NJ):
                    for h in range(HPC):
                        pts[c][(h, jb)] = scores_exp(c, jb, h)
                    if jb < NJ // 2:
                        pv_part(2 * jb, pts[c - 1], acc, sums)
                        pv_part(2 * jb + 1, pts[c - 1], acc, sums)
                    elif c == NI - 1:
                        if jb == NJ // 2:
                            acc7 = aux_ps.tile([128, IC], f32, name="acc",
                                               tag="aux")
                            sums7 = aux_ps.tile([128, 16], f32, name="sums",
                                                tag="aux")
                        gg = jb - NJ // 2
                        pv_part(2 * gg, pts[c], acc7, sums7)
                        pv_part(2 * gg + 1, pts[c], acc7, sums7)
                    if jb == NJ // 2 - 1:
                        norm_transpose(c - 1, acc, sums)
                        pts[c - 1] = None
            norm_transpose(NI - 1, acc7, sums7)

            # --- reshard + output projection ---
            for u in range(2):
                if single:
                    nc.sync.dma_start(out=a2a_out[u][:, :], in_=a2a_in[u][:, :])
                else:
                    nc.gpsimd.collective_compute(
                        "AllToAll", mybir.AluOpType.bypass,
                        replica_groups=[list(range(N_CORES))],
                        ins=[a2a_in[u].opt()], outs=[a2a_out[u].opt()])
            for u in range(2):
                for r in range(8):
                    nc.sync.dma_start(
                        out=go_sb[:, IC * r + IC // 2 * u:
                                  IC * r + IC // 2 * (u + 1)],
                        in_=a2a_out[u][CPC * r:CPC * (r + 1), :])
            for ib in range(4):
                for e in range(2):
                    y_ps = sc_ps.tile([128, IC], f32, name="y_ps", tag="sc")
                    for t in range(8):
                        nc.tensor.matmul(
                            y_ps[:, :],
                            go_sb[:, IC * t + JB * ib:IC * t + JB * (ib + 1)],
                            wo_sb[:, QD * t + IC * e:QD * t + IC * (e + 1)],
                            start=(t == 0), stop=(t == 7))
                    y_sb = y_pool.tile([128, IC], bf16, name="y_sb", tag="ysb")
                    nc.vector.tensor_copy(y_sb[:, :], y_ps[:, :])
                    nc.sync.dma_start(
                        out=y_d[JB * ib:JB * (ib + 1), IC * e:IC * (e + 1)],
                        in_=y_sb[:, :])
    nc.compile()
    return nc


def _get_nc():
    if "nc" not in _CACHE:
        _CACHE["nc"] = _build()
    return _CACHE["nc"]


def make_in_maps(x, Wq, Wk, Wv, Wo):
    import ml_dtypes
    bf = ml_dtypes.bfloat16
    xt = np.ascontiguousarray(x.reshape(N, QD).T.astype(bf))
    wo = np.ascontiguousarray(Wo.astype(bf))

    def sbuf_layout(w):
        # [1024, 128] -> [128 p, 8 t x 128 c] matching wq_sb resident layout
        return np.ascontiguousarray(
            w.astype(bf).reshape(8, 128, CPC).transpose(1, 0, 2)
            .reshape(128, 8 * CPC))

    in_maps = []
    for k in range(N_CORES):
        cs = CPC * k
        in_maps.append({
            "xt": xt,
            "wq": sbuf_layout(Wq[:, cs:cs + CPC]),
            "wk": sbuf_layout(Wk[:, cs:cs + CPC]),
            "wv": sbuf_layout(Wv[:, cs:cs + CPC]),
            "wo": wo,
        })
    return in_maps


def kernel(x, Wq, Wk, Wv, Wo, bo):
    from concourse.bass_utils import run_bass_kernel_spmd

    x = np.asarray(x, dtype=np.float32)
    Wq = np.asarray(Wq, dtype=np.float32)
    Wk = np.asarray(Wk, dtype=np.float32)
    Wv = np.asarray(Wv, dtype=np.float32)
    Wo = np.asarray(Wo, dtype=np.float32)
    bo = np.asarray(bo, dtype=np.float32)

    nc = _get_nc()
    in_maps = make_in_maps(x, Wq, Wk, Wv, Wo)
    res = run_bass_kernel_spmd(nc, in_maps, list(range(N_CORES)))
    y = np.concatenate(
        [np.asarray(res.results[k]["y_out"], dtype=np.float32)
         for k in range(N_CORES)], axis=0)
    y = y + bo[None, :]
    return y.reshape(1, N, QD).astype(np.float32)



# revision 20
# speedup vs baseline: 1.0441x; 1.0441x over previous
"""Multi-head cross-attention (self-attention variant) on 8 Trainium2 NeuronCores.

Problem: x[1,4096,1024]; Wq/Wk/Wv[1024,1024] -> 16 heads x 64 dim; softmax(QK^T/8)V;
merge heads; @ Wo + bo -> [1,4096,1024].

Sharding: tensor-parallel over heads. Core k owns heads (2k, 2k+1) = inner cols
[128k : 128k+128]. All activations/weights are bf16 (measured rel err ~9e-3 vs
the 2e-2 gate), which keeps every matmul at 1 PE cycle/row at any free size.

Per core:
  - Q^T/K^T [128, 4096] projected chunk-wise (contraction = model dim, moving = x^T).
  - V projected directly in [j, d] layout (stationary = x^T tile, moving = Wv);
    4 j-block slots share one PSUM bank as a single accumulation group (a
    start=True matmul pend-zeroes the whole 2KB bank; later start=False slots
    land on pending-zero bytes and accumulate from zero).
  - Scores S^T[j, i] per (head, j-block): stat = K^T block, mov = Q^T; each
    [128, 512] f32 output exactly fills one PSUM bank. Six banks rotate as
    score buffers -- the deep window keeps the PE->exp->PE loop from being
    latency-bound (2-bank tiles with a 3-deep window cost ~25% throughput).
  - exp: softmax without max-subtraction (logits ~ N(0,1), exp is safe).
    Split 34/30 between the scalar engine (activation Exp, bf16 out) and DVE
    (Schraudolph fast exp2: bf16(exp(s)) bits == int16(s*FE_A + FE_B), one
    fused tensor_scalar writing int16 -- the DVE write-converter truncates,
    centered by +0.5 in FE_B; int16 range is safe for |logit| << 64 sigma).
    GPSIMD cannot read PSUM (BIR verifier), so it takes no exp work.
  - PV transposed: stat = P^T block [j, 128 i], mov = V [j, 64 d] -> O[i, d],
    64 rows/matmul instead of 512 (2x fewer PE rows than O^T = V^T P). All 8
    (i-block, head) slots accumulate in ONE bank as a single group; row-sums
    accumulate in a second bank via 1-row matmuls against a ones vector.
  - Normalize: DVE reciprocal of the 8 sums, then ONE broadcast tensor_tensor
    multiply (stride-0 AP spreads rcp[i, slot] over each slot's 64 dims).
  - O [i, d] -> O^T via DMA xbar transpose (dma_start_transpose: no PE, PSUM
    or DVE cost). Two half-width AllToAlls reshard head-parallel ->
    sequence-parallel (the second half's exchange overlaps the first half's
    go loads and output-proj matmuls); core k ends with rows [512k : 512k+512]
    of the merged-head activation and applies the full Wo; host concatenates
    row slices and adds bo.

Emission order software-pipelines the in-order PE queue: chunk c's scores
interleave with chunk c-1's PV at 2 PV-parts per j-block for the first half
of the chunk (PV finishes mid-chunk, giving normalize a long window before
the accumulator bank is reused); chunk 7's own PV runs in its second half.
Projections for chunks 1-7 are woven into chunk 0's score stream.
"""
import numpy as np
from contextlib import ExitStack

N_CORES = 8
N = 4096          # sequence length
QD = 1024         # model dim
DH = 64           # head dim
HPC = 2           # heads per core
CPC = HPC * DH    # inner cols per core = 128
IC = 512          # i-chunk (query) size
NI = N // IC      # 8 chunks
JB = 128          # j-block (key) size
NJ = N // JB      # 32 blocks
NG = 16           # j-groups per chunk (2 j-blocks each)
SCALE = DH ** -0.5

# fast-exp routing per chunk: r = 2*jb + h in [0, 64). Fast tiles run ONE
# fused DVE op: tensor_scalar(s*A + B) written straight to int16 (the DVE
# write-converter truncates, which the +0.5 in FE_B centers); the rest use
# the scalar engine's Exp. GPSIMD cannot touch PSUM, so it takes no exp work.
FE_DVE = frozenset(r for r in range(64) if r % 2 == 1) - {15, 47}
# last chunk: r=62 stays on Act so the final two exps (r=62 Act, r=63 DVE)
# drain in parallel rather than serializing on DVE
FE_DVE_LAST = FE_DVE
# bf16 bits of exp(s*SCALE) ~= int16(s*FE_A + FE_B):
#   FE_A = 2^23/(ln2 * 2^16) * SCALE,  FE_B = 127*128 - C/2^16 (+0.5 trunc bias)
FE_A = 184.6638356 * SCALE
FE_B = 16249.066

WARMUP_MM = 9    # [1,512] dummies bridging the startup DMA wait
TAIL_MM = 56     # [1,256] dummies bridging pair 3's reshard chain

_CACHE = {}


def _build(single=False):
    from concourse import bacc, tile, mybir

    f32 = mybir.dt.float32
    bf16 = mybir.dt.bfloat16
    i16 = mybir.dt.int16
    Exp = mybir.ActivationFunctionType.Exp
    Mult = mybir.AluOpType.mult
    Add = mybir.AluOpType.add
    Div = mybir.AluOpType.divide

    nc = bacc.Bacc("TRN2", target_bir_lowering=False, debug=False,
                   enable_asserts=False, num_devices=1 if single else N_CORES)

    xt_d = nc.dram_tensor("xt", [QD, N], bf16, kind="ExternalInput").ap()
    # wq/wk/wv arrive host-pre-arranged in SBUF layout [p, (t c)] so the
    # startup loads are contiguous 2KB-row copies (256B runs pay a 2x DMA
    # latency multiplier)
    wq_d = nc.dram_tensor("wq", [128, 8 * CPC], bf16, kind="ExternalInput").ap()
    wk_d = nc.dram_tensor("wk", [128, 8 * CPC], bf16, kind="ExternalInput").ap()
    wv_d = nc.dram_tensor("wv", [128, 8 * CPC], bf16, kind="ExternalInput").ap()
    wo_d = nc.dram_tensor("wo", [QD, QD], bf16, kind="ExternalInput").ap()
    # y rows are (pair p, 128-row block): core k holds global rows
    # 1024*p + 128*k .. +128 for p in 0..4
    y_d = nc.dram_tensor("y_out", [IC, QD], bf16, kind="ExternalOutput").ap()

    with tile.TileContext(nc) as tc:
        with ExitStack() as ctx:
            sb = ctx.enter_context(tc.tile_pool(name="sb", bufs=1))
            xt_pool = ctx.enter_context(tc.tile_pool(name="xt", bufs=3))
            pt_pool = ctx.enter_context(tc.tile_pool(name="pt", bufs=76))
            o_pool = ctx.enter_context(tc.tile_pool(name="osb", bufs=3))
            ot_pool = ctx.enter_context(tc.tile_pool(name="ot", bufs=2))
            sm_pool = ctx.enter_context(tc.tile_pool(name="sm", bufs=2))
            y_pool = ctx.enter_context(tc.tile_pool(name="ysb", bufs=8))
            sc_ps = ctx.enter_context(tc.tile_pool(name="sc", bufs=6, space="PSUM"))
            aux_ps = ctx.enter_context(tc.tile_pool(name="aux", bufs=2, space="PSUM"))
            dram = ctx.enter_context(tc.tile_pool(name="dram", bufs=1, space="DRAM"))

            # --- static SBUF residents ---
            qts = [sb.tile([CPC, IC], bf16, name=f"qt{c}") for c in range(NI)]
            kts = [sb.tile([CPC, IC], bf16, name=f"kt{c}") for c in range(NI)]
            # vs[c]: V[j, d] for j-block 4c+b at cols [128b : 128b+128]
            vs = [sb.tile([128, IC], bf16, name=f"v{c}") for c in range(NI)]
            wq_sb = sb.tile([128, 8 * CPC], bf16)   # qd-tile t at cols 128t
            wk_sb = sb.tile([128, 8 * CPC], bf16)
            wv_sb = sb.tile([128, 8 * CPC], bf16)
            wo_sb = sb.tile([128, 8 * QD], bf16)    # qd-tile t at cols 1024t
            # per-pair a2a results: dims-tile m at cols 128m
            go_sb = [sb.tile([128, N_CORES * JB], bf16, name=f"go{p}")
                     for p in range(NI // 2)]
            ones_sb = sb.tile([128, 1], bf16)
            warm_sb = sb.tile([128, IC], bf16)

            # reshard per chunk-PAIR (1024 i): row-block b of a2a_in[p] is
            # i-block b (128 i) of the pair with this core's 128 dims; the
            # AllToAll lands block m (core m's dims for MY i-block) at
            # a2a_out[p] rows 128m. Pairs 0-2 exchange mid-stream, fully
            # overlapped; only pair 3's exchange is tail-exposed.
            a2a_in = [dram.tile([N_CORES * CPC, JB], bf16,
                                name=f"a2a_in{p}") for p in range(NI // 2)]
            a2a_out = [dram.tile([N_CORES * CPC, JB], bf16,
                                 name=f"a2a_out{p}") for p in range(NI // 2)]

            nc.vector.memset(ones_sb[:, :], 1.0)

            def load_w(dst, src):
                nc.sync.dma_start(out=dst[:, :], in_=src)

            def load_wo(dst, src):
                nc.sync.dma_start(
                    out=dst[:, :].rearrange("p (t c) -> p t c", t=8),
                    in_=src.rearrange("(t p) c -> p t c", t=8))

            def load_xt(c):
                xt_c = xt_pool.tile([128, 8 * IC], bf16, name=f"xt{c}", tag="xt")
                nc.sync.dma_start(
                    out=xt_c[:, :].rearrange("p (t i) -> p t i", t=8),
                    in_=xt_d.rearrange("(t p) n -> p t n", t=8)[:, :, IC * c:IC * (c + 1)])
                return xt_c

            def proj_qk(c, xt_c):
                q_ps = aux_ps.tile([128, IC], f32, name="q_ps", tag="aux")
                for t in range(8):
                    nc.tensor.matmul(q_ps[:, :], wq_sb[:, CPC * t:CPC * (t + 1)],
                                     xt_c[:, IC * t:IC * (t + 1)],
                                     start=(t == 0), stop=(t == 7))
                nc.vector.tensor_copy(qts[c][:, :], q_ps[:, :])
                k_ps = aux_ps.tile([128, IC], f32, name="k_ps", tag="aux")
                for t in range(8):
                    nc.tensor.matmul(k_ps[:, :], wk_sb[:, CPC * t:CPC * (t + 1)],
                                     xt_c[:, IC * t:IC * (t + 1)],
                                     start=(t == 0), stop=(t == 7))
                nc.vector.tensor_copy(kts[c][:, :], k_ps[:, :])

            def proj_v(c, xt_c):
                # V in [j, d]: stat = x^T tile (qd x j), mov = Wv tile (qd x d);
                # 4 j-block slots in one bank, single accumulation group
                v_ps = aux_ps.tile([128, IC], f32, name="v_ps", tag="aux")
                for b in range(4):
                    for t in range(8):
                        nc.tensor.matmul(
                            v_ps[:, JB * b:JB * (b + 1)],
                            xt_c[:, IC * t + JB * b:IC * t + JB * (b + 1)],
                            wv_sb[:, CPC * t:CPC * (t + 1)],
                            start=(b == 0 and t == 0), stop=(b == 3 and t == 7))
                nc.vector.tensor_copy(vs[c][:, :], v_ps[:, :])

            def scores_exp(c, jb, h):
                r = 2 * jb + h
                fe_dve = FE_DVE_LAST if c == NI - 1 else FE_DVE
                s_ps = sc_ps.tile([128, IC], f32, name="s_ps", tag="sc")
                nc.tensor.matmul(
                    s_ps[:, :],
                    kts[jb // 4][DH * h:DH * (h + 1),
                                 JB * (jb % 4):JB * (jb % 4 + 1)],
                    qts[c][DH * h:DH * (h + 1), :], start=True, stop=True)
                pt = pt_pool.tile([128, IC], bf16, name="pt", tag="pt")
                if r in fe_dve:
                    nc.vector.tensor_scalar(out=pt[:, :].bitcast(i16),
                                            in0=s_ps[:, :],
                                            scalar1=float(FE_A),
                                            scalar2=float(FE_B),
                                            op0=Mult, op1=Add)
                else:
                    nc.scalar.activation(pt[:, :], s_ps[:, :], Exp, scale=SCALE)
                return pt

            def pv_part(jb, pts_c, acc, sums):
                # consume pt tiles of j-block jb: O[i, d] and sum rows
                g4, b = jb // 4, jb % 4
                for ib in range(4):
                    for h in range(HPC):
                        first = (jb == 0 and ib == 0 and h == 0)
                        last = (jb == NJ - 1 and ib == 3 and h == 1)
                        stat = pts_c[(h, jb)][:, JB * ib:JB * (ib + 1)]
                        s = 2 * ib + h
                        nc.tensor.matmul(
                            acc[:, DH * s:DH * (s + 1)], stat,
                            vs[g4][:, JB * b + DH * h:JB * b + DH * (h + 1)],
                            start=first, stop=last)
                        nc.tensor.matmul(
                            sums[:, s:s + 1], stat, ones_sb[:, :],
                            start=first, stop=last)

            def norm_transpose(c, acc, sums):
                # O * (1/sum(exp)) on DVE (the only vector engine allowed to
                # read PSUM): reciprocal of the 8 sums, then per-partition
                # scalar multiplies
                rcp = sm_pool.tile([128, 8], f32, name="rcp", tag="sm")
                nc.vector.reciprocal(rcp[:, :], sums[:, 0:8])
                otT = ot_pool.tile([128, IC], bf16, name="otT", tag="ot")
                # one broadcast multiply normalizes all 8 (i-block, head)
                # slots: rcp[i, s] spreads over each slot's 64 dims via a
                # stride-0 AP
                o_sb = o_pool.tile([128, IC], bf16, name="osb", tag="osb")
                nc.vector.tensor_mul(
                    o_sb[:, :].rearrange("p (s d) -> p s d", s=8),
                    acc[:, :].rearrange("p (s d) -> p s d", s=8),
                    rcp[:, :].to_broadcast([128, 8, DH]))
                for ib in range(4):
                    # the last chunk's transposes are tail-exposed: split
                    # them across the SP and Act HWDGE queues
                    eng = nc.scalar if (c == NI - 1 and ib % 2) else nc.sync
                    eng.dma_start_transpose(
                        out=otT[:, JB * ib:JB * (ib + 1)],
                        in_=o_sb[:, JB * ib:JB * (ib + 1)])
                # i-block b of this chunk -> a2a_in[c//2] row-block (4*(c%2)+b)
                nc.sync.dma_start(
                    out=a2a_in[c // 2][IC * (c % 2):IC * (c % 2 + 1), :]
                        .rearrange("(b p) i -> p b i", b=4),
                    in_=otT[:, :].rearrange("p (b i) -> p b i", b=4))

            def exchange(p):
                if single:
                    nc.sync.dma_start(out=a2a_out[p][:, :], in_=a2a_in[p][:, :])
                else:
                    nc.gpsimd.collective_compute(
                        "AllToAll", mybir.AluOpType.bypass,
                        replica_groups=[list(range(N_CORES))],
                        ins=[a2a_in[p].opt()], outs=[a2a_out[p].opt()])
                nc.sync.dma_start(
                    out=go_sb[p][:, :].rearrange("p (m i) -> p m i", m=8),
                    in_=a2a_out[p][:, :].rearrange("(m p) i -> p m i", m=8))

            def outproj(p):
                for e in range(2):
                    # score banks are idle at the tail; avoids waiting on
                    # acc7/sums7 release in the aux pool
                    y_ps = sc_ps.tile([128, IC], f32, name="y_ps", tag="sc")
                    for m in range(8):
                        nc.tensor.matmul(
                            y_ps[:, :], go_sb[p][:, JB * m:JB * (m + 1)],
                            wo_sb[:, QD * m + IC * e:QD * m + IC * (e + 1)],
                            start=(m == 0), stop=(m == 7))
                    y_sb = y_pool.tile([128, IC], bf16, name="y_sb", tag="ysb")
                    nc.vector.tensor_copy(y_sb[:, :], y_ps[:, :])
                    # y writes ride SWDGE (gpsimd) to keep the SP queue free
                    # for the latency-critical reshard chain
                    nc.gpsimd.dma_start(
                        out=y_d[JB * p:JB * (p + 1), IC * e:IC * (e + 1)],
                        in_=y_sb[:, :])

            # --- emission ---
            # warm_sb filled by the (otherwise idle) gpsimd engine in
            # parallel with the DVE ones memset, so warmup matmuls can
            # start within ~0.5us
            nc.gpsimd.memset(warm_sb[:, :], 1.0)
            load_w(wq_sb, wq_d)
            xt0 = xt_pool.tile([128, 8 * IC], bf16, name="xt0", tag="xt")
            for t in range(8):
                nc.sync.dma_start(
                    out=xt0[:, IC * t:IC * (t + 1)],
                    in_=xt_d[128 * t:128 * (t + 1), 0:IC])
            load_w(wk_sb, wk_d)
            load_w(wv_sb, wv_d)
            # warmup matmuls: keep the PE busy through the startup DMA wait
            # so the p-state ramp completes before the first projection
            warm_ps = aux_ps.tile([128, IC], f32, name="warm", tag="aux")
            for _ in range(WARMUP_MM):
                nc.tensor.matmul(warm_ps[0:1, :], ones_sb[:, :],
                                 warm_sb[:, :], start=True, stop=True)
            proj_qk(0, xt0)
            proj_v(0, xt0)

            pts = [dict() for _ in range(NI)]
            # chunk 0 scores interleaved with remaining projections; V lags
            # Q/K by two j-blocks so its PSUM slot reuse never stalls PE
            xts = {}
            for jb in range(NJ):
                if jb % 4 == 0 and jb // 4 + 1 < NI:
                    m = jb // 4 + 1
                    xts[m] = load_xt(m)
                    proj_qk(m, xts[m])
                if jb % 4 == 2 and jb // 4 + 1 < NI:
                    m = jb // 4 + 1
                    proj_v(m, xts[m])
                    del xts[m]
                for h in range(HPC):
                    pts[0][(h, jb)] = scores_exp(0, jb, h)
            load_wo(wo_sb, wo_d)
            # steady: scores(c) interleave with PV(c-1) at double rate in the
            # first half of each chunk, so normalize(c-1) (gpsimd) has a long
            # window before PV(c) reuses the accumulator bank. Chunk 7's own
            # PV runs in its second half, right behind its exps.
            for c in range(1, NI):
                acc = aux_ps.tile([128, IC], f32, name="acc", tag="aux")
                sums = aux_ps.tile([128, 16], f32, name="sums", tag="aux")
                for jb in range(NJ):
                    for h in range(HPC):
                        pts[c][(h, jb)] = scores_exp(c, jb, h)
                    if jb < NJ // 2:
                        pv_part(2 * jb, pts[c - 1], acc, sums)
                        pv_part(2 * jb + 1, pts[c - 1], acc, sums)
                    elif c == NI - 1:
                        if jb == NJ // 2:
                            acc7 = aux_ps.tile([128, IC], f32, name="acc",
                                               tag="aux")
                            sums7 = aux_ps.tile([128, 16], f32, name="sums",
                                                tag="aux")
                        gg = jb - NJ // 2
                        pv_part(2 * gg, pts[c], acc7, sums7)
                        pv_part(2 * gg + 1, pts[c], acc7, sums7)
                    if jb == NJ // 2 - 1:
                        norm_transpose(c - 1, acc, sums)
                        pts[c - 1] = None
                        if (c - 1) % 2 == 1:
                            exchange((c - 1) // 2)
            # all four output projections run at the tail: pairs 0-2 have
            # long-resident go data, so their 48 matmuls keep the PE busy
            # (warm) while pair 3's norm -> reshard DMA chain drains; pair
            # 3's matmuls start the moment its go tile lands. Filler
            # matmuls bridge the remaining chain latency so outproj(3)
            # never sees a p-state reset.
            norm_transpose(NI - 1, acc7, sums7)
            exchange(NI // 2 - 1)
            for p in range(NI // 2 - 1):
                outproj(p)
            tail_ps = sc_ps.tile([128, IC], f32, name="tail_ps", tag="sc")
            for _ in range(TAIL_MM):
                nc.tensor.matmul(tail_ps[0:1, 0:256], ones_sb[:, :],
                                 warm_sb[:, 0:256], start=True, stop=True)
            outproj(NI // 2 - 1)
    nc.compile()
    return nc


def _get_nc():
    if "nc" not in _CACHE:
        _CACHE["nc"] = _build()
    return _CACHE["nc"]


def make_in_maps(x, Wq, Wk, Wv, Wo):
    import ml_dtypes
    bf = ml_dtypes.bfloat16
    xt = np.ascontiguousarray(x.reshape(N, QD).T.astype(bf))
    wo = np.ascontiguousarray(Wo.astype(bf))

    def sbuf_layout(w):
        # [1024, 128] -> [128 p, 8 t x 128 c] matching wq_sb resident layout
        return np.ascontiguousarray(
            w.astype(bf).reshape(8, 128, CPC).transpose(1, 0, 2)
            .reshape(128, 8 * CPC))

    in_maps = []
    for k in range(N_CORES):
        cs = CPC * k
        in_maps.append({
            "xt": xt,
            "wq": sbuf_layout(Wq[:, cs:cs + CPC]),
            "wk": sbuf_layout(Wk[:, cs:cs + CPC]),
            "wv": sbuf_layout(Wv[:, cs:cs + CPC]),
            "wo": wo,
        })
    return in_maps


def kernel(x, Wq, Wk, Wv, Wo, bo):
    from concourse.bass_utils import run_bass_kernel_spmd

    x = np.asarray(x, dtype=np.float32)
    Wq = np.asarray(Wq, dtype=np.float32)
    Wk = np.asarray(Wk, dtype=np.float32)
    Wv = np.asarray(Wv, dtype=np.float32)
    Wo = np.asarray(Wo, dtype=np.float32)
    bo = np.asarray(bo, dtype=np.float32)

    nc = _get_nc()
    in_maps = make_in_maps(x, Wq, Wk, Wv, Wo)
    res = run_bass_kernel_spmd(nc, in_maps, list(range(N_CORES)))
    # core k's y rows are (pair p, 128): global rows 1024p + 128k .. +128
    yk = np.stack([np.asarray(res.results[k]["y_out"], dtype=np.float32)
                   for k in range(N_CORES)])          # [8, 512, 1024]
    y = yk.reshape(N_CORES, 4, JB, QD).transpose(1, 0, 2, 3).reshape(N, QD)
    y = y + bo[None, :]
    return y.reshape(1, N, QD).astype(np.float32)



# revision 34
# speedup vs baseline: 1.0937x; 1.0475x over previous
"""Multi-head cross-attention (self-attention variant) on 8 Trainium2 NeuronCores.

Problem: x[1,4096,1024]; Wq/Wk/Wv[1024,1024] -> 16 heads x 64 dim; softmax(QK^T/8)V;
merge heads; @ Wo + bo -> [1,4096,1024].

Sharding: tensor-parallel over heads. Core k owns heads (2k, 2k+1) = inner cols
[128k : 128k+128]. All activations/weights are bf16 (measured rel err ~9e-3 vs
the 2e-2 gate), which keeps every matmul at 1 PE cycle/row at any free size.

Per core:
  - Q^T/K^T [128, 4096] projected chunk-wise (contraction = model dim, moving = x^T).
  - V projected directly in [j, d] layout (stationary = x^T tile, moving = Wv);
    4 j-block slots share one PSUM bank as a single accumulation group (a
    start=True matmul pend-zeroes the whole 2KB bank; later start=False slots
    land on pending-zero bytes and accumulate from zero).
  - Scores S^T[j, i] per (head, j-block): stat = K^T block, mov = Q^T; each
    [128, 512] f32 output exactly fills one PSUM bank. Six banks rotate as
    score buffers -- the deep window keeps the PE->exp->PE loop from being
    latency-bound (2-bank tiles with a 3-deep window cost ~25% throughput).
  - exp: softmax without max-subtraction (logits ~ N(0,1), exp is safe).
    Split 34/30 between the scalar engine (activation Exp, bf16 out) and DVE
    (Schraudolph fast exp2: bf16(exp(s)) bits == int16(s*FE_A + FE_B), one
    fused tensor_scalar writing int16 -- the DVE write-converter truncates,
    centered by +0.5 in FE_B; int16 range is safe for |logit| << 64 sigma).
    GPSIMD cannot read PSUM (BIR verifier), so it takes no exp work.
  - PV transposed: stat = P^T block [j, 128 i], mov = V [j, 64 d] -> O[i, d],
    64 rows/matmul instead of 512 (2x fewer PE rows than O^T = V^T P). All 8
    (i-block, head) slots accumulate in ONE bank as a single group; row-sums
    accumulate in a second bank via 1-row matmuls against a ones vector.
  - Normalize: DVE reciprocal of the 8 sums, then ONE broadcast tensor_tensor
    multiply (stride-0 AP spreads rcp[i, slot] over each slot's 64 dims).
  - O [i, d] -> O^T via DMA xbar transpose (dma_start_transpose: no PE, PSUM
    or DVE cost). Two half-width AllToAlls reshard head-parallel ->
    sequence-parallel (the second half's exchange overlaps the first half's
    go loads and output-proj matmuls); core k ends with rows [512k : 512k+512]
    of the merged-head activation and applies the full Wo; host concatenates
    row slices and adds bo.

Emission order software-pipelines the in-order PE queue: chunk c's scores
interleave with chunk c-1's PV at 2 PV-parts per j-block for the first half
of the chunk (PV finishes mid-chunk, giving normalize a long window before
the accumulator bank is reused); chunk 7's own PV runs in its second half.
Projections for chunks 1-7 are woven into chunk 0's score stream.
"""
import numpy as np
from contextlib import ExitStack

N_CORES = 8
N = 4096          # sequence length
QD = 1024         # model dim
DH = 64           # head dim
HPC = 2           # heads per core
CPC = HPC * DH    # inner cols per core = 128
IC = 512          # i-chunk (query) size
NI = N // IC      # 8 chunks
JB = 128          # j-block (key) size
NJ = N // JB      # 32 blocks
NG = 16           # j-groups per chunk (2 j-blocks each)
SCALE = DH ** -0.5

# fast-exp routing per chunk: r = 2*jb + h in [0, 64). Fast tiles run ONE
# fused DVE op: tensor_scalar(s*A + B) written straight to int16 (the DVE
# write-converter truncates, which the +0.5 in FE_B centers); the rest use
# the scalar engine's Exp. GPSIMD cannot touch PSUM, so it takes no exp work.
FE_DVE = frozenset(r for r in range(64) if r % 2 == 1) - {15, 47}
# last chunk: r=62 stays on Act so the final two exps (r=62 Act, r=63 DVE)
# drain in parallel rather than serializing on DVE
FE_DVE_LAST = FE_DVE
# bf16 bits of exp(s*SCALE) ~= int16(s*FE_A + FE_B):
#   FE_A = 2^23/(ln2 * 2^16) * SCALE,  FE_B = 127*128 - C/2^16 (+0.5 trunc bias)
FE_A = 184.6638356 * SCALE
FE_B = 16249.066

WARMUP_MM = 9    # [1,512] dummies bridging the startup DMA wait
TAIL_MM = 56     # [1,256] dummies bridging pair 3's reshard chain
SW = 1024.0      # fp8 pre-scale on W (hi+lo land in e4m3 normal range)
SX = 32.0        # fp8 pre-scale on x^T
PSCALE = SW * SX

_CACHE = {}


def _build(single=False):
    from concourse import bacc, tile, mybir

    f32 = mybir.dt.float32
    bf16 = mybir.dt.bfloat16
    i16 = mybir.dt.int16
    Exp = mybir.ActivationFunctionType.Exp
    Mult = mybir.AluOpType.mult
    Add = mybir.AluOpType.add
    Div = mybir.AluOpType.divide

    nc = bacc.Bacc("TRN2", target_bir_lowering=False, debug=False,
                   enable_asserts=False, num_devices=1 if single else N_CORES)

    f8 = mybir.dt.float8e4
    DR = mybir.MatmulPerfMode.DoubleRow

    # x^T and the q/k/v weights arrive as fp8 hi/lo pairs (a = a1 + a2, both
    # e4m3). Operands are pre-scaled by powers of 2 (W by 1024, x by 32) so
    # both hi values AND residuals land in e4m3's normal range (raw W ~0.03
    # and the residuals would otherwise die in subnormals); the product
    # carries a uniform 2^15 that the PSUM->SBUF copy divides back out.
    # Projections run as DoubleRow fp8 matmuls (2 k-tiles per instruction,
    # half cycles/row) accumulating w1x1 + w1x2 + w2x1; the dropped w2x2
    # term plus residual quantization is ~0.1% rms.
    xt1_d = nc.dram_tensor("xt1", [QD, N], f8, kind="ExternalInput").ap()
    xt2_d = nc.dram_tensor("xt2", [QD, N], f8, kind="ExternalInput").ap()
    w_d = {(nm, h): nc.dram_tensor(f"{nm}{h}", [128, 8 * CPC], f8,
                                   kind="ExternalInput").ap()
           for nm in ("wq", "wk", "wv") for h in (1, 2)}
    wo_d = nc.dram_tensor("wo", [QD, QD], bf16, kind="ExternalInput").ap()
    # y rows are (pair p, 128-row block): core k holds global rows
    # 1024*p + 128*k .. +128 for p in 0..4
    y_d = nc.dram_tensor("y_out", [IC, QD], bf16, kind="ExternalOutput").ap()

    with tile.TileContext(nc) as tc:
        with ExitStack() as ctx:
            sb = ctx.enter_context(tc.tile_pool(name="sb", bufs=1))
            xt_pool = ctx.enter_context(tc.tile_pool(name="xt", bufs=6))
            pt_pool = ctx.enter_context(tc.tile_pool(name="pt", bufs=76))
            o_pool = ctx.enter_context(tc.tile_pool(name="osb", bufs=3))
            ot_pool = ctx.enter_context(tc.tile_pool(name="ot", bufs=2))
            sm_pool = ctx.enter_context(tc.tile_pool(name="sm", bufs=2))
            y_pool = ctx.enter_context(tc.tile_pool(name="ysb", bufs=8))
            sc_ps = ctx.enter_context(tc.tile_pool(name="sc", bufs=6, space="PSUM"))
            aux_ps = ctx.enter_context(tc.tile_pool(name="aux", bufs=2, space="PSUM"))
            dram = ctx.enter_context(tc.tile_pool(name="dram", bufs=1, space="DRAM"))

            # --- static SBUF residents ---
            qts = [sb.tile([CPC, IC], bf16, name=f"qt{c}") for c in range(NI)]
            kts = [sb.tile([CPC, IC], bf16, name=f"kt{c}") for c in range(NI)]
            # vs[c]: V[j, d] for j-block 4c+b at cols [128b : 128b+128]
            vs = [sb.tile([128, IC], bf16, name=f"v{c}") for c in range(NI)]
            # fp8 hi/lo weight residents, qd-tile t at cols 128t
            w_sb = {(nm, h): sb.tile([128, 8 * CPC], f8, name=f"{nm}{h}")
                    for nm in ("wq", "wk", "wv") for h in (1, 2)}
            wo_sb = sb.tile([128, 8 * QD], bf16)    # qd-tile t at cols 1024t
            # per-pair a2a results: dims-tile m at cols 128m
            go_sb = [sb.tile([128, N_CORES * JB], bf16, name=f"go{p}")
                     for p in range(NI // 2)]
            ones_sb = sb.tile([128, 1], bf16)
            warm_sb = sb.tile([128, IC], bf16)

            # reshard per chunk-PAIR (1024 i): row-block b of a2a_in[p] is
            # i-block b (128 i) of the pair with this core's 128 dims; the
            # AllToAll lands block m (core m's dims for MY i-block) at
            # a2a_out[p] rows 128m. Pairs 0-2 exchange mid-stream, fully
            # overlapped; only pair 3's exchange is tail-exposed.
            a2a_in = [dram.tile([N_CORES * CPC, JB], bf16,
                                name=f"a2a_in{p}") for p in range(NI // 2)]
            a2a_out = [dram.tile([N_CORES * CPC, JB], bf16,
                                 name=f"a2a_out{p}") for p in range(NI // 2)]

            nc.vector.memset(ones_sb[:, :], 1.0)

            def load_w(dst, src):
                nc.sync.dma_start(out=dst[:, :], in_=src)

            def load_wo(dst, src):
                nc.sync.dma_start(
                    out=dst[:, :].rearrange("p (t c) -> p t c", t=8),
                    in_=src.rearrange("(t p) c -> p t c", t=8))

            def load_xt(c):
                xs = []
                for i, src in enumerate((xt1_d, xt2_d)):
                    x_c = xt_pool.tile([128, 8 * IC], f8,
                                       name=f"xt{c}_{i}", tag="xt")
                    nc.sync.dma_start(
                        out=x_c[:, :].rearrange("p (t i) -> p t i", t=8),
                        in_=src.rearrange("(t p) n -> p t n", t=8)
                        [:, :, IC * c:IC * (c + 1)])
                    xs.append(x_c)
                return tuple(xs)

            def proj_one(name, w1, w2, x1, x2, dst):
                # 12 DoubleRow matmuls: k-tile pairs (2j, 2j+1), passes
                # (w1 x1, w1 x2, w2 x1) per pair; one accumulation group
                ps = aux_ps.tile([128, IC], f32, name=name, tag="aux")
                n = 0
                for j in range(4):
                    for wt, xt_c in ((w1, x1), (w1, x2), (w2, x1)):
                        nc.tensor.matmul(
                            ps[:, :],
                            wt[:, :].rearrange("p (t c) -> p t c", t=8)
                            [:, 2 * j:2 * j + 2, :],
                            xt_c[:, :].rearrange("p (t i) -> p t i", t=8)
                            [:, 2 * j:2 * j + 2, :],
                            start=(n == 0), stop=(n == 11), perf_mode=DR)
                        n += 1
                nc.vector.tensor_scalar(out=dst[:, :], in0=ps[:, :],
                                        scalar1=1.0 / PSCALE, scalar2=None,
                                        op0=Mult)

            def proj_qk(c, x1, x2):
                proj_one("q_ps", w_sb["wq", 1], w_sb["wq", 2], x1, x2, qts[c])
                proj_one("k_ps", w_sb["wk", 1], w_sb["wk", 2], x1, x2, kts[c])

            def proj_v(c, x1, x2):
                # V in [j, d]: stat = x^T k-tile pair (qd x j), mov = Wv
                # pair; 4 j-block slots in one bank, single accumulation
                # group of 48 DoubleRow matmuls
                v_ps = aux_ps.tile([128, IC], f32, name="v_ps", tag="aux")
                n = 0
                for b in range(4):
                    for j in range(4):
                        for xt_c, wt in ((x1, w_sb["wv", 1]),
                                         (x2, w_sb["wv", 1]),
                                         (x1, w_sb["wv", 2])):
                            nc.tensor.matmul(
                                v_ps[:, JB * b:JB * (b + 1)],
                                xt_c[:, :].rearrange("p (t i) -> p t i", t=8)
                                [:, 2 * j:2 * j + 2, JB * b:JB * (b + 1)],
                                wt[:, :].rearrange("p (t c) -> p t c", t=8)
                                [:, 2 * j:2 * j + 2, :],
                                start=(n == 0), stop=(n == 47), perf_mode=DR)
                            n += 1
                nc.vector.tensor_scalar(out=vs[c][:, :], in0=v_ps[:, :],
                                        scalar1=1.0 / PSCALE, scalar2=None,
                                        op0=Mult)

            def scores_exp(c, jb, h):
                r = 2 * jb + h
                fe_dve = FE_DVE_LAST if c == NI - 1 else FE_DVE
                s_ps = sc_ps.tile([128, IC], f32, name="s_ps", tag="sc")
                nc.tensor.matmul(
                    s_ps[:, :],
                    kts[jb // 4][DH * h:DH * (h + 1),
                                 JB * (jb % 4):JB * (jb % 4 + 1)],
                    qts[c][DH * h:DH * (h + 1), :], start=True, stop=True)
                pt = pt_pool.tile([128, IC], bf16, name="pt", tag="pt")
                if r in fe_dve:
                    nc.vector.tensor_scalar(out=pt[:, :].bitcast(i16),
                                            in0=s_ps[:, :],
                                            scalar1=float(FE_A),
                                            scalar2=float(FE_B),
                                            op0=Mult, op1=Add)
                else:
                    nc.scalar.activation(pt[:, :], s_ps[:, :], Exp, scale=SCALE)
                return pt

            def pv_part(jb, pts_c, acc, sums):
                # consume pt tiles of j-block jb: O[i, d] and sum rows
                g4, b = jb // 4, jb % 4
                for ib in range(4):
                    for h in range(HPC):
                        first = (jb == 0 and ib == 0 and h == 0)
                        last = (jb == NJ - 1 and ib == 3 and h == 1)
                        stat = pts_c[(h, jb)][:, JB * ib:JB * (ib + 1)]
                        s = 2 * ib + h
                        nc.tensor.matmul(
                            acc[:, DH * s:DH * (s + 1)], stat,
                            vs[g4][:, JB * b + DH * h:JB * b + DH * (h + 1)],
                            start=first, stop=last)
                        nc.tensor.matmul(
                            sums[:, s:s + 1], stat, ones_sb[:, :],
                            start=first, stop=last)

            def norm_transpose(c, acc, sums):
                # O * (1/sum(exp)) on DVE (the only vector engine allowed to
                # read PSUM): reciprocal of the 8 sums, then per-partition
                # scalar multiplies
                rcp = sm_pool.tile([128, 8], f32, name="rcp", tag="sm")
                nc.vector.reciprocal(rcp[:, :], sums[:, 0:8])
                otT = ot_pool.tile([128, IC], bf16, name="otT", tag="ot")
                # one broadcast multiply normalizes all 8 (i-block, head)
                # slots: rcp[i, s] spreads over each slot's 64 dims via a
                # stride-0 AP
                o_sb = o_pool.tile([128, IC], bf16, name="osb", tag="osb")
                nc.vector.tensor_mul(
                    o_sb[:, :].rearrange("p (s d) -> p s d", s=8),
                    acc[:, :].rearrange("p (s d) -> p s d", s=8),
                    rcp[:, :].to_broadcast([128, 8, DH]))
                for ib in range(4):
                    # the last chunk's transposes are tail-exposed: split
                    # them across the SP and Act HWDGE queues
                    eng = nc.scalar if (c == NI - 1 and ib % 2) else nc.sync
                    eng.dma_start_transpose(
                        out=otT[:, JB * ib:JB * (ib + 1)],
                        in_=o_sb[:, JB * ib:JB * (ib + 1)])
                # i-block b of this chunk -> a2a_in[c//2] row-block (4*(c%2)+b)
                nc.sync.dma_start(
                    out=a2a_in[c // 2][IC * (c % 2):IC * (c % 2 + 1), :]
                        .rearrange("(b p) i -> p b i", b=4),
                    in_=otT[:, :].rearrange("p (b i) -> p b i", b=4))

            def exchange(p):
                if single:
                    nc.sync.dma_start(out=a2a_out[p][:, :], in_=a2a_in[p][:, :])
                else:
                    nc.gpsimd.collective_compute(
                        "AllToAll", mybir.AluOpType.bypass,
                        replica_groups=[list(range(N_CORES))],
                        ins=[a2a_in[p].opt()], outs=[a2a_out[p].opt()])
                nc.sync.dma_start(
                    out=go_sb[p][:, :].rearrange("p (m i) -> p m i", m=8),
                    in_=a2a_out[p][:, :].rearrange("(m p) i -> p m i", m=8))

            def outproj(p):
                for e in range(2):
                    # score banks are idle at the tail; avoids waiting on
                    # acc7/sums7 release in the aux pool
                    y_ps = sc_ps.tile([128, IC], f32, name="y_ps", tag="sc")
                    for m in range(8):
                        nc.tensor.matmul(
                            y_ps[:, :], go_sb[p][:, JB * m:JB * (m + 1)],
                            wo_sb[:, QD * m + IC * e:QD * m + IC * (e + 1)],
                            start=(m == 0), stop=(m == 7))
                    y_sb = y_pool.tile([128, IC], bf16, name="y_sb", tag="ysb")
                    nc.vector.tensor_copy(y_sb[:, :], y_ps[:, :])
                    # y writes ride SWDGE (gpsimd) to keep the SP queue free
                    # for the latency-critical reshard chain
                    nc.gpsimd.dma_start(
                        out=y_d[JB * p:JB * (p + 1), IC * e:IC * (e + 1)],
                        in_=y_sb[:, :])

            # --- emission ---
            # warm_sb filled by the (otherwise idle) gpsimd engine in
            # parallel with the DVE ones memset, so warmup matmuls can
            # start within ~0.5us
            nc.gpsimd.memset(warm_sb[:, :], 1.0)
            for h in (1, 2):
                load_w(w_sb["wq", h], w_d["wq", h])
            # chunk 0's x^T lands as 4 interleaved hi/lo half-loads so the
            # first projection matmuls start as early as possible
            xt0_1 = xt_pool.tile([128, 8 * IC], f8, name="xt0_1", tag="xt")
            xt0_2 = xt_pool.tile([128, 8 * IC], f8, name="xt0_2", tag="xt")
            for half in range(2):
                for x_c, src in ((xt0_1, xt1_d), (xt0_2, xt2_d)):
                    nc.sync.dma_start(
                        out=x_c[:, 4 * IC * half:4 * IC * (half + 1)]
                            .rearrange("p (t i) -> p t i", t=4),
                        in_=src.rearrange("(t p) n -> p t n", t=8)
                        [:, 4 * half:4 * (half + 1), 0:IC])
            for nm in ("wk", "wv"):
                for h in (1, 2):
                    load_w(w_sb[nm, h], w_d[nm, h])
            # warmup matmuls: keep the PE busy through the startup DMA wait
            # so the p-state ramp completes before the first projection
            warm_ps = aux_ps.tile([128, IC], f32, name="warm", tag="aux")
            for _ in range(WARMUP_MM):
                nc.tensor.matmul(warm_ps[0:1, :], ones_sb[:, :],
                                 warm_sb[:, :], start=True, stop=True)
            proj_qk(0, xt0_1, xt0_2)
            proj_v(0, xt0_1, xt0_2)

            pts = [dict() for _ in range(NI)]
            # chunk 0 scores interleaved with remaining projections; V lags
            # Q/K by two j-blocks so its PSUM slot reuse never stalls PE
            xts = {}
            for jb in range(NJ):
                if jb % 4 == 0 and jb // 4 + 1 < NI:
                    m = jb // 4 + 1
                    xts[m] = load_xt(m)
                    proj_qk(m, *xts[m])
                if jb % 4 == 2 and jb // 4 + 1 < NI:
                    m = jb // 4 + 1
                    proj_v(m, *xts[m])
                    del xts[m]
                for h in range(HPC):
                    pts[0][(h, jb)] = scores_exp(0, jb, h)
            load_wo(wo_sb, wo_d)
            # steady: scores(c) interleave with PV(c-1) at double rate in the
            # first half of each chunk, so normalize(c-1) (gpsimd) has a long
            # window before PV(c) reuses the accumulator bank. Chunk 7's own
            # PV runs in its second half, right behind its exps.
            for c in range(1, NI):
                acc = aux_ps.tile([128, IC], f32, name="acc", tag="aux")
                sums = aux_ps.tile([128, 16], f32, name="sums", tag="aux")
                for jb in range(NJ):
                    for h in range(HPC):
                        pts[c][(h, jb)] = scores_exp(c, jb, h)
                    if jb < NJ // 2:
                        pv_part(2 * jb, pts[c - 1], acc, sums)
                        pv_part(2 * jb + 1, pts[c - 1], acc, sums)
                    elif c == NI - 1:
                        if jb == NJ // 2:
                            acc7 = aux_ps.tile([128, IC], f32, name="acc",
                                               tag="aux")
                            sums7 = aux_ps.tile([128, 16], f32, name="sums",
                                                tag="aux")
                        gg = jb - NJ // 2
                        pv_part(2 * gg, pts[c], acc7, sums7)
                        pv_part(2 * gg + 1, pts[c], acc7, sums7)
                    if jb == NJ // 2 - 1:
                        norm_transpose(c - 1, acc, sums)
                        pts[c - 1] = None
                        if (c - 1) % 2 == 1:
                            exchange((c - 1) // 2)
            # all four output projections run at the tail: pairs 0-2 have
            # long-resident go data, so their 48 matmuls keep the PE busy
            # (warm) while pair 3's norm -> reshard DMA chain drains; pair
            # 3's matmuls start the moment its go tile lands. Filler
            # matmuls bridge the remaining chain latency so outproj(3)
            # never sees a p-state reset.
            norm_transpose(NI - 1, acc7, sums7)
            exchange(NI // 2 - 1)
            for p in range(NI // 2 - 1):
                outproj(p)
            tail_ps = sc_ps.tile([128, IC], f32, name="tail_ps", tag="sc")
            for _ in range(TAIL_MM):
                nc.tensor.matmul(tail_ps[0:1, 0:256], ones_sb[:, :],
                                 warm_sb[:, 0:256], start=True, stop=True)
            outproj(NI // 2 - 1)
    nc.compile()
    return nc


def _get_nc():
    if "nc" not in _CACHE:
        _CACHE["nc"] = _build()
    return _CACHE["nc"]


def make_in_maps(x, Wq, Wk, Wv, Wo):
    import ml_dtypes
    bf = ml_dtypes.bfloat16
    f8 = ml_dtypes.float8_e4m3

    def split8(a, s):
        a = (a * s).astype(np.float32)
        hi = a.astype(f8)
        lo = (a - hi.astype(np.float32)).astype(f8)
        return hi, lo

    xt = np.ascontiguousarray(x.reshape(N, QD).T)
    xt1, xt2 = split8(xt, SX)
    wo = np.ascontiguousarray(Wo.astype(bf))

    def sbuf_layout(w):
        # [1024, 128] -> [128 p, 8 t x 128 c] matching the resident layout
        return np.ascontiguousarray(
            w.reshape(8, 128, CPC).transpose(1, 0, 2).reshape(128, 8 * CPC))

    in_maps = []
    for k in range(N_CORES):
        cs = CPC * k
        m = {"xt1": xt1, "xt2": xt2, "wo": wo}
        for nm, w in (("wq", Wq), ("wk", Wk), ("wv", Wv)):
            hi, lo = split8(np.ascontiguousarray(w[:, cs:cs + CPC]), SW)
            m[nm + "1"] = sbuf_layout(hi)
            m[nm + "2"] = sbuf_layout(lo)
        in_maps.append(m)
    return in_maps


def kernel(x, Wq, Wk, Wv, Wo, bo):
    from concourse.bass_utils import run_bass_kernel_spmd

    x = np.asarray(x, dtype=np.float32)
    Wq = np.asarray(Wq, dtype=np.float32)
    Wk = np.asarray(Wk, dtype=np.float32)
    Wv = np.asarray(Wv, dtype=np.float32)
    Wo = np.asarray(Wo, dtype=np.float32)
    bo = np.asarray(bo, dtype=np.float32)

    nc = _get_nc()
    in_maps = make_in_maps(x, Wq, Wk, Wv, Wo)
    res = run_bass_kernel_spmd(nc, in_maps, list(range(N_CORES)))
    # core k's y rows are (pair p, 128): global rows 1024p + 128k .. +128
    yk = np.stack([np.asarray(res.results[k]["y_out"], dtype=np.float32)
                   for k in range(N_CORES)])          # [8, 512, 1024]
    y = yk.reshape(N_CORES, 4, JB, QD).transpose(1, 0, 2, 3).reshape(N, QD)
    y = y + bo[None, :]
    return y.reshape(1, N, QD).astype(np.float32)



# revision 53
# speedup vs baseline: 1.1060x; 1.0113x over previous
"""Multi-head cross-attention (self-attention variant) on 8 Trainium2 NeuronCores.

Problem: x[1,4096,1024]; Wq/Wk/Wv[1024,1024] -> 16 heads x 64 dim; softmax(QK^T/8)V;
merge heads; @ Wo + bo -> [1,4096,1024].

Sharding: tensor-parallel over heads. Core k owns heads (2k, 2k+1) = inner cols
[128k : 128k+128]. All activations/weights are bf16 (measured rel err ~9e-3 vs
the 2e-2 gate), which keeps every matmul at 1 PE cycle/row at any free size.

Per core:
  - Q^T/K^T [128, 4096] projected chunk-wise (contraction = model dim, moving = x^T).
  - V projected directly in [j, d] layout (stationary = x^T tile, moving = Wv);
    4 j-block slots share one PSUM bank as a single accumulation group (a
    start=True matmul pend-zeroes the whole 2KB bank; later start=False slots
    land on pending-zero bytes and accumulate from zero).
  - Scores S^T[j, i] per (head, j-block): stat = K^T block, mov = Q^T; each
    [128, 512] f32 output exactly fills one PSUM bank. Six banks rotate as
    score buffers -- the deep window keeps the PE->exp->PE loop from being
    latency-bound (2-bank tiles with a 3-deep window cost ~25% throughput).
  - exp: softmax without max-subtraction (logits ~ N(0,1), exp is safe).
    Split 34/30 between the scalar engine (activation Exp, bf16 out) and DVE
    (Schraudolph fast exp2: bf16(exp(s)) bits == int16(s*FE_A + FE_B), one
    fused tensor_scalar writing int16 -- the DVE write-converter truncates,
    centered by +0.5 in FE_B; int16 range is safe for |logit| << 64 sigma).
    GPSIMD cannot read PSUM (BIR verifier), so it takes no exp work.
  - PV transposed: stat = P^T block [j, 128 i], mov = V [j, 64 d] -> O[i, d],
    64 rows/matmul instead of 512 (2x fewer PE rows than O^T = V^T P). All 8
    (i-block, head) slots accumulate in ONE bank as a single group; row-sums
    accumulate in a second bank via 1-row matmuls against a ones vector.
  - Normalize: DVE reciprocal of the 8 sums, then ONE broadcast tensor_tensor
    multiply (stride-0 AP spreads rcp[i, slot] over each slot's 64 dims).
  - O [i, d] -> O^T via DMA xbar transpose (dma_start_transpose: no PE, PSUM
    or DVE cost). Two half-width AllToAlls reshard head-parallel ->
    sequence-parallel (the second half's exchange overlaps the first half's
    go loads and output-proj matmuls); core k ends with rows [512k : 512k+512]
    of the merged-head activation and applies the full Wo; host concatenates
    row slices and adds bo.

Emission order software-pipelines the in-order PE queue: chunk c's scores
interleave with chunk c-1's PV at 2 PV-parts per j-block for the first half
of the chunk (PV finishes mid-chunk, giving normalize a long window before
the accumulator bank is reused); chunk 7's own PV runs in its second half.
Projections for chunks 1-7 are woven into chunk 0's score stream.
"""
import numpy as np
from contextlib import ExitStack

N_CORES = 8
N = 4096          # sequence length
QD = 1024         # model dim
DH = 64           # head dim
HPC = 2           # heads per core
CPC = HPC * DH    # inner cols per core = 128
IC = 512          # i-chunk (query) size
NI = N // IC      # 8 chunks
JB = 128          # j-block (key) size
NJ = N // JB      # 32 blocks
NG = 16           # j-groups per chunk (2 j-blocks each)
SCALE = DH ** -0.5

# fast-exp routing per chunk: r = 2*jb + h in [0, 64). Fast tiles run ONE
# fused DVE op: tensor_scalar(s*A + B) written straight to int16 (the DVE
# write-converter truncates, which the +0.5 in FE_B centers); the rest use
# the scalar engine's Exp. GPSIMD cannot touch PSUM, so it takes no exp work.
FE_DVE = frozenset(r for r in range(64) if r % 2 == 1) - {15, 47}
# last chunk: r=62 stays on Act so the final two exps (r=62 Act, r=63 DVE)
# drain in parallel rather than serializing on DVE
FE_DVE_LAST = FE_DVE
# bf16 bits of exp(s*SCALE) ~= int16(s*FE_A + FE_B):
#   FE_A = 2^23/(ln2 * 2^16) * SCALE,  FE_B = 127*128 - C/2^16 (+0.5 trunc bias)
FE_A = 184.6638356 * SCALE
FE_B = 16249.066

WARMUP_MM = 9    # [1,512] dummies bridging the startup DMA wait
TAIL_MM = 36     # [1,256] dummies bridging pair 3's reshard chain
SW = 1024.0      # fp8 pre-scale on W (hi+lo land in e4m3 normal range)
SX = 32.0        # fp8 pre-scale on x^T
PSCALE = SW * SX

_CACHE = {}


def _build(single=False):
    from concourse import bacc, tile, mybir

    f32 = mybir.dt.float32
    bf16 = mybir.dt.bfloat16
    i16 = mybir.dt.int16
    Exp = mybir.ActivationFunctionType.Exp
    Mult = mybir.AluOpType.mult
    Add = mybir.AluOpType.add
    Div = mybir.AluOpType.divide

    nc = bacc.Bacc("TRN2", target_bir_lowering=False, debug=False,
                   enable_asserts=False, num_devices=1 if single else N_CORES)

    f8 = mybir.dt.float8e4
    DR = mybir.MatmulPerfMode.DoubleRow

    # x^T and the q/k/v weights arrive as fp8 hi/lo pairs (a = a1 + a2, both
    # e4m3). Operands are pre-scaled by powers of 2 (W by 1024, x by 32) so
    # both hi values AND residuals land in e4m3's normal range (raw W ~0.03
    # and the residuals would otherwise die in subnormals); the product
    # carries a uniform 2^15 that the PSUM->SBUF copy divides back out.
    # Projections run as DoubleRow fp8 matmuls (2 k-tiles per instruction,
    # half cycles/row) accumulating w1x1 + w1x2 + w2x1; the dropped w2x2
    # term plus residual quantization is ~0.1% rms.
    xt1_d = nc.dram_tensor("xt1", [QD, N], f8, kind="ExternalInput").ap()
    xt2_d = nc.dram_tensor("xt2", [QD, N], f8, kind="ExternalInput").ap()
    w_d = {(nm, h): nc.dram_tensor(f"{nm}{h}", [128, 8 * CPC], f8,
                                   kind="ExternalInput").ap()
           for nm in ("wq", "wk", "wv") for h in (1, 2)}
    wo_d = nc.dram_tensor("wo", [QD, QD], bf16, kind="ExternalInput").ap()
    # y rows are (pair p, 128-row block): core k holds global rows
    # 1024*p + 128*k .. +128 for p in 0..4
    y_d = nc.dram_tensor("y_out", [IC, QD], bf16, kind="ExternalOutput").ap()

    with tile.TileContext(nc) as tc:
        with ExitStack() as ctx:
            sb = ctx.enter_context(tc.tile_pool(name="sb", bufs=1))
            xt_pool = ctx.enter_context(tc.tile_pool(name="xt", bufs=6))
            pt_pool = ctx.enter_context(tc.tile_pool(name="pt", bufs=76))
            o_pool = ctx.enter_context(tc.tile_pool(name="osb", bufs=3))
            ot_pool = ctx.enter_context(tc.tile_pool(name="ot", bufs=2))
            sm_pool = ctx.enter_context(tc.tile_pool(name="sm", bufs=2))
            y_pool = ctx.enter_context(tc.tile_pool(name="ysb", bufs=8))
            sc_ps = ctx.enter_context(tc.tile_pool(name="sc", bufs=6, space="PSUM"))
            aux_ps = ctx.enter_context(tc.tile_pool(name="aux", bufs=2, space="PSUM"))
            dram = ctx.enter_context(tc.tile_pool(name="dram", bufs=1, space="DRAM"))

            # --- static SBUF residents ---
            qts = [sb.tile([CPC, IC], bf16, name=f"qt{c}") for c in range(NI)]
            kts = [sb.tile([CPC, IC], bf16, name=f"kt{c}") for c in range(NI)]
            # vs[c]: V[j, d] for j-block 4c+b at cols [128b : 128b+128]
            vs = [sb.tile([128, IC], bf16, name=f"v{c}") for c in range(NI)]
            # fp8 hi/lo weight residents, qd-tile t at cols 128t
            w_sb = {(nm, h): sb.tile([128, 8 * CPC], f8, name=f"{nm}{h}")
                    for nm in ("wq", "wk", "wv") for h in (1, 2)}
            wo_sb = sb.tile([128, 8 * QD], bf16)    # qd-tile t at cols 1024t
            # per-pair a2a results: dims-tile m at cols 128m
            go_sb = [sb.tile([128, N_CORES * JB], bf16, name=f"go{p}")
                     for p in range(NI // 2)]
            ones_sb = sb.tile([128, 1], bf16)

            # reshard per chunk-PAIR (1024 i): row-block b of a2a_in[p] is
            # i-block b (128 i) of the pair with this core's 128 dims; the
            # AllToAll lands block m (core m's dims for MY i-block) at
            # a2a_out[p] rows 128m. Pairs 0-2 exchange mid-stream, fully
            # overlapped; only pair 3's exchange is tail-exposed.
            a2a_in = [dram.tile([N_CORES * CPC, JB], bf16,
                                name=f"a2a_in{p}") for p in range(NI // 2)]
            a2a_out = [dram.tile([N_CORES * CPC, JB], bf16,
                                 name=f"a2a_out{p}") for p in range(NI // 2)]

            nc.vector.memset(ones_sb[:, :], 1.0)

            def load_w(dst, src):
                nc.sync.dma_start(out=dst[:, :], in_=src)

            def load_wo(dst, src):
                nc.sync.dma_start(
                    out=dst[:, :].rearrange("p (t c) -> p t c", t=8),
                    in_=src.rearrange("(t p) c -> p t c", t=8))

            def load_xt(c):
                xs = []
                for i, src in enumerate((xt1_d, xt2_d)):
                    x_c = xt_pool.tile([128, 8 * IC], f8,
                                       name=f"xt{c}_{i}", tag="xt")
                    nc.sync.dma_start(
                        out=x_c[:, :].rearrange("p (t i) -> p t i", t=8),
                        in_=src.rearrange("(t p) n -> p t n", t=8)
                        [:, :, IC * c:IC * (c + 1)])
                    xs.append(x_c)
                return tuple(xs)

            def proj_one(name, w1, w2, x1, x2, dst):
                # 12 DoubleRow matmuls: k-tile pairs (2j, 2j+1), passes
                # (w1 x1, w1 x2, w2 x1) per pair; one accumulation group
                ps = aux_ps.tile([128, IC], f32, name=name, tag="aux")
                n = 0
                for j in range(4):
                    for wt, xt_c in ((w1, x1), (w1, x2), (w2, x1)):
                        nc.tensor.matmul(
                            ps[:, :],
                            wt[:, :].rearrange("p (t c) -> p t c", t=8)
                            [:, 2 * j:2 * j + 2, :],
                            xt_c[:, :].rearrange("p (t i) -> p t i", t=8)
                            [:, 2 * j:2 * j + 2, :],
                            start=(n == 0), stop=(n == 11), perf_mode=DR)
                        n += 1
                nc.vector.tensor_scalar(out=dst[:, :], in0=ps[:, :],
                                        scalar1=1.0 / PSCALE, scalar2=None,
                                        op0=Mult)

            def proj_qk(c, x1, x2):
                proj_one("q_ps", w_sb["wq", 1], w_sb["wq", 2], x1, x2, qts[c])
                proj_one("k_ps", w_sb["wk", 1], w_sb["wk", 2], x1, x2, kts[c])

            def proj_v(c, x1, x2):
                # V in [j, d]: stat = x^T k-tile pair (qd x j), mov = Wv
                # pair; 4 j-block slots in one bank, single accumulation
                # group of 48 DoubleRow matmuls
                v_ps = aux_ps.tile([128, IC], f32, name="v_ps", tag="aux")
                n = 0
                for b in range(4):
                    for j in range(4):
                        for xt_c, wt in ((x1, w_sb["wv", 1]),
                                         (x2, w_sb["wv", 1]),
                                         (x1, w_sb["wv", 2])):
                            nc.tensor.matmul(
                                v_ps[:, JB * b:JB * (b + 1)],
                                xt_c[:, :].rearrange("p (t i) -> p t i", t=8)
                                [:, 2 * j:2 * j + 2, JB * b:JB * (b + 1)],
                                wt[:, :].rearrange("p (t c) -> p t c", t=8)
                                [:, 2 * j:2 * j + 2, :],
                                start=(n == 0), stop=(n == 47), perf_mode=DR)
                            n += 1
                nc.vector.tensor_scalar(out=vs[c][:, :], in0=v_ps[:, :],
                                        scalar1=1.0 / PSCALE, scalar2=None,
                                        op0=Mult)

            def scores_exp(c, jb, h):
                r = 2 * jb + h
                fe_dve = FE_DVE_LAST if c == NI - 1 else FE_DVE
                s_ps = sc_ps.tile([128, IC], f32, name="s_ps", tag="sc")
                nc.tensor.matmul(
                    s_ps[:, :],
                    kts[jb // 4][DH * h:DH * (h + 1),
                                 JB * (jb % 4):JB * (jb % 4 + 1)],
                    qts[c][DH * h:DH * (h + 1), :], start=True, stop=True)
                pt = pt_pool.tile([128, IC], bf16, name="pt", tag="pt")
                if r in fe_dve:
                    nc.vector.tensor_scalar(out=pt[:, :].bitcast(i16),
                                            in0=s_ps[:, :],
                                            scalar1=float(FE_A),
                                            scalar2=float(FE_B),
                                            op0=Mult, op1=Add)
                else:
                    nc.scalar.activation(pt[:, :], s_ps[:, :], Exp, scale=SCALE)
                return pt

            def pv_part(jb, pts_c, acc, sums):
                # consume pt tiles of j-block jb: O[i, d] and sum rows
                g4, b = jb // 4, jb % 4
                for ib in range(4):
                    for h in range(HPC):
                        first = (jb == 0 and ib == 0 and h == 0)
                        last = (jb == NJ - 1 and ib == 3 and h == 1)
                        stat = pts_c[(h, jb)][:, JB * ib:JB * (ib + 1)]
                        s = 2 * ib + h
                        nc.tensor.matmul(
                            acc[:, DH * s:DH * (s + 1)], stat,
                            vs[g4][:, JB * b + DH * h:JB * b + DH * (h + 1)],
                            start=first, stop=last)
                        nc.tensor.matmul(
                            sums[:, s:s + 1], stat, ones_sb[:, :],
                            start=first, stop=last)

            def norm_transpose(c, acc, sums):
                # O * (1/sum(exp)) on DVE (the only vector engine allowed to
                # read PSUM): reciprocal of the 8 sums, then per-partition
                # scalar multiplies
                rcp = sm_pool.tile([128, 8], f32, name="rcp", tag="sm")
                nc.vector.reciprocal(rcp[:, :], sums[:, 0:8])
                otT = ot_pool.tile([128, IC], bf16, name="otT", tag="ot")
                # one broadcast multiply normalizes all 8 (i-block, head)
                # slots: rcp[i, s] spreads over each slot's 64 dims via a
                # stride-0 AP
                o_sb = o_pool.tile([128, IC], bf16, name="osb", tag="osb")
                nc.vector.tensor_mul(
                    o_sb[:, :].rearrange("p (s d) -> p s d", s=8),
                    acc[:, :].rearrange("p (s d) -> p s d", s=8),
                    rcp[:, :].to_broadcast([128, 8, DH]))
                for ib in range(4):
                    # the last chunk's transposes are tail-exposed: split
                    # them across the SP and Act HWDGE queues
                    eng = nc.scalar if (c == NI - 1 and ib % 2) else nc.sync
                    eng.dma_start_transpose(
                        out=otT[:, JB * ib:JB * (ib + 1)],
                        in_=o_sb[:, JB * ib:JB * (ib + 1)])
                # i-block b of this chunk -> a2a_in[c//2] row-block (4*(c%2)+b)
                nc.sync.dma_start(
                    out=a2a_in[c // 2][IC * (c % 2):IC * (c % 2 + 1), :]
                        .rearrange("(b p) i -> p b i", b=4),
                    in_=otT[:, :].rearrange("p (b i) -> p b i", b=4))

            def exchange(p, split=False):
                if single:
                    # equivalent DRAM copy standing in for the AllToAll; the
                    # tail pair splits it in half so the go loads pipeline
                    # behind the copy halves
                    for u in range(2 if split else 1):
                        sl = slice(512 * u, 512 * (u + 1) if split else 1024)
                        nc.sync.dma_start(out=a2a_out[p][sl, :],
                                          in_=a2a_in[p][sl, :])
                else:
                    nc.gpsimd.collective_compute(
                        "AllToAll", mybir.AluOpType.bypass,
                        replica_groups=[list(range(N_CORES))],
                        ins=[a2a_in[p].opt()], outs=[a2a_out[p].opt()])
                for u in range(2 if split else 1):
                    sl = slice(4 * u, 4 * (u + 1) if split else 8)
                    nc.sync.dma_start(
                        out=go_sb[p][:, 512 * u:512 * (u + 1) if split else 1024]
                            .rearrange("p (m i) -> p m i", m=4 if split else 8),
                        in_=a2a_out[p][:, :]
                            .rearrange("(m p) i -> p m i", m=8)[:, sl, :])

            def outproj(p, split=False):
                for e in range(2):
                    # score banks are idle at the tail; avoids waiting on
                    # acc7/sums7 release in the aux pool
                    y_ps = sc_ps.tile([128, IC], f32, name="y_ps", tag="sc")
                    for m in range(8):
                        nc.tensor.matmul(
                            y_ps[:, :], go_sb[p][:, JB * m:JB * (m + 1)],
                            wo_sb[:, QD * m + IC * e:QD * m + IC * (e + 1)],
                            start=(m == 0), stop=(m == 7))
                    y_sb = y_pool.tile([128, IC], bf16, name="y_sb", tag="ysb")
                    # the last pair's writes are the kernel's last mile:
                    # split them so the DMA halves pipeline behind the
                    # copies, with the halves on DVE and Act in parallel
                    for u in range(2 if split else 1):
                        sl = slice(256 * u, 256 * (u + 1) if split else IC)
                        if split and u == 1:
                            nc.scalar.activation(
                                y_sb[:, sl], y_ps[:, sl],
                                mybir.ActivationFunctionType.Copy)
                        else:
                            nc.vector.tensor_copy(y_sb[:, sl], y_ps[:, sl])
                        # y writes ride SWDGE (gpsimd) to keep the SP queue
                        # free for the latency-critical reshard chain; the
                        # split halves go down both paths in parallel
                        eng = nc.sync if (split and u == 1) else nc.gpsimd
                        eng.dma_start(
                            out=y_d[JB * p:JB * (p + 1), IC * e + 256 * u:
                                    IC * e + (256 * (u + 1) if split else IC)],
                            in_=y_sb[:, sl])

            # --- emission ---
            # warmup matmuls first: they read (uninitialized) resident
            # tiles, so they have no dependencies and start within ~150ns,
            # keeping the PE busy through the startup DMA wait so the
            # p-state ramp completes before the first projection
            warm_ps = aux_ps.tile([128, IC], f32, name="warm", tag="aux")
            for _ in range(WARMUP_MM):
                nc.tensor.matmul(warm_ps[0:1, :], qts[0][:, 0:1],
                                 qts[0][:, :], start=True, stop=True)
            # chunk 0's x^T lands as 4 interleaved hi/lo half-loads,
            # sequenced so each projection's operands land just in time
            xt0_1 = xt_pool.tile([128, 8 * IC], f8, name="xt0_1", tag="xt")
            xt0_2 = xt_pool.tile([128, 8 * IC], f8, name="xt0_2", tag="xt")

            def load_xt0(x_c, src, half):
                nc.sync.dma_start(
                    out=x_c[:, 4 * IC * half:4 * IC * (half + 1)]
                        .rearrange("p (t i) -> p t i", t=4),
                    in_=src.rearrange("(t p) n -> p t n", t=8)
                    [:, 4 * half:4 * (half + 1), 0:IC])

            load_w(w_sb["wq", 1], w_d["wq", 1])
            load_w(w_sb["wq", 2], w_d["wq", 2])
            load_xt0(xt0_1, xt1_d, 0)
            load_xt0(xt0_2, xt2_d, 0)
            load_xt0(xt0_1, xt1_d, 1)
            load_xt0(xt0_2, xt2_d, 1)
            load_w(w_sb["wk", 1], w_d["wk", 1])
            load_w(w_sb["wk", 2], w_d["wk", 2])
            load_w(w_sb["wv", 1], w_d["wv", 1])
            load_w(w_sb["wv", 2], w_d["wv", 2])
            proj_qk(0, xt0_1, xt0_2)
            proj_v(0, xt0_1, xt0_2)

            pts = [dict() for _ in range(NI)]
            # chunk 0 scores interleaved with remaining projections; V lags
            # Q/K by two j-blocks so its PSUM slot reuse never stalls PE
            xts = {}
            for jb in range(NJ):
                if jb % 4 == 0 and jb // 4 + 1 < NI:
                    m = jb // 4 + 1
                    xts[m] = load_xt(m)
                    proj_qk(m, *xts[m])
                if jb % 4 == 2 and jb // 4 + 1 < NI:
                    m = jb // 4 + 1
                    proj_v(m, *xts[m])
                    del xts[m]
                for h in range(HPC):
                    pts[0][(h, jb)] = scores_exp(0, jb, h)
            load_wo(wo_sb, wo_d)
            # steady: scores(c) interleave with PV(c-1) at double rate in the
            # first half of each chunk, so normalize(c-1) (gpsimd) has a long
            # window before PV(c) reuses the accumulator bank. Chunk 7's own
            # PV runs in its second half, right behind its exps.
            for c in range(1, NI):
                acc = aux_ps.tile([128, IC], f32, name="acc", tag="aux")
                sums = aux_ps.tile([128, 16], f32, name="sums", tag="aux")
                for jb in range(NJ):
                    for h in range(HPC):
                        pts[c][(h, jb)] = scores_exp(c, jb, h)
                    if jb < NJ // 2:
                        pv_part(2 * jb, pts[c - 1], acc, sums)
                        pv_part(2 * jb + 1, pts[c - 1], acc, sums)
                    elif c == NI - 1:
                        if jb == NJ // 2:
                            acc7 = aux_ps.tile([128, IC], f32, name="acc",
                                               tag="aux")
                            sums7 = aux_ps.tile([128, 16], f32, name="sums",
                                                tag="aux")
                        gg = jb - NJ // 2
                        pv_part(2 * gg, pts[c], acc7, sums7)
                        pv_part(2 * gg + 1, pts[c], acc7, sums7)
                    if jb == NJ // 2 - 1:
                        norm_transpose(c - 1, acc, sums)
                        pts[c - 1] = None
                        if (c - 1) % 2 == 1:
                            exchange((c - 1) // 2)
            # all four output projections run at the tail: pairs 0-2 have
            # long-resident go data, so their 48 matmuls keep the PE busy
            # (warm) while pair 3's norm -> reshard DMA chain drains; pair
            # 3's matmuls start the moment its go tile lands. Filler
            # matmuls bridge the remaining chain latency so outproj(3)
            # never sees a p-state reset.
            norm_transpose(NI - 1, acc7, sums7)
            exchange(NI // 2 - 1, split=True)
            for p in range(NI // 2 - 1):
                outproj(p)
            tail_ps = sc_ps.tile([128, IC], f32, name="tail_ps", tag="sc")
            for _ in range(TAIL_MM):
                nc.tensor.matmul(tail_ps[0:1, 0:256], qts[0][:, 0:1],
                                 qts[0][:, 0:256], start=True, stop=True)
            outproj(NI // 2 - 1, split=True)
    nc.compile()
    return nc


def _get_nc():
    if "nc" not in _CACHE:
        _CACHE["nc"] = _build()
    return _CACHE["nc"]


def make_in_maps(x, Wq, Wk, Wv, Wo):
    import ml_dtypes
    bf = ml_dtypes.bfloat16
    f8 = ml_dtypes.float8_e4m3

    def split8(a, s):
        a = (a * s).astype(np.float32)
        hi = a.astype(f8)
        lo = (a - hi.astype(np.float32)).astype(f8)
        return hi, lo

    xt = np.ascontiguousarray(x.reshape(N, QD).T)
    xt1, xt2 = split8(xt, SX)
    wo = np.ascontiguousarray(Wo.astype(bf))

    def sbuf_layout(w):
        # [1024, 128] -> [128 p, 8 t x 128 c] matching the resident layout
        return np.ascontiguousarray(
            w.reshape(8, 128, CPC).transpose(1, 0, 2).reshape(128, 8 * CPC))

    in_maps = []
    for k in range(N_CORES):
        cs = CPC * k
        m = {"xt1": xt1, "xt2": xt2, "wo": wo}
        for nm, w in (("wq", Wq), ("wk", Wk), ("wv", Wv)):
            hi, lo = split8(np.ascontiguousarray(w[:, cs:cs + CPC]), SW)
            m[nm + "1"] = sbuf_layout(hi)
            m[nm + "2"] = sbuf_layout(lo)
        in_maps.append(m)
    return in_maps


def kernel(x, Wq, Wk, Wv, Wo, bo):
    from concourse.bass_utils import run_bass_kernel_spmd

    x = np.asarray(x, dtype=np.float32)
    Wq = np.asarray(Wq, dtype=np.float32)
    Wk = np.asarray(Wk, dtype=np.float32)
    Wv = np.asarray(Wv, dtype=np.float32)
    Wo = np.asarray(Wo, dtype=np.float32)
    bo = np.asarray(bo, dtype=np.float32)

    nc = _get_nc()
    in_maps = make_in_maps(x, Wq, Wk, Wv, Wo)
    res = run_bass_kernel_spmd(nc, in_maps, list(range(N_CORES)))
    # core k's y rows are (pair p, 128): global rows 1024p + 128k .. +128
    yk = np.stack([np.asarray(res.results[k]["y_out"], dtype=np.float32)
                   for k in range(N_CORES)])          # [8, 512, 1024]
    y = yk.reshape(N_CORES, 4, JB, QD).transpose(1, 0, 2, 3).reshape(N, QD)
    y = y + bo[None, :]
    return y.reshape(1, N, QD).astype(np.float32)

